# revision 1
# baseline (speedup 1.0000x reference)
"""Multi-head attention with additive positional attention — TRN2 Bass kernel.

Problem: B=4, S=2048, DM=128, H=8, DK=16.
  scores = (q @ k^T) / sqrt(DK) + pos_q @ pos_k^T   per (b, h)
  out    = softmax(scores) @ v, heads merged, @ Wo^T + bo

Sharding: 8 cores = batch (4) x query-row halves (2). Each core holds one
batch's full keys/values (S=2048) and 1024 query rows, computes all 8 heads,
and produces complete output rows — no cross-core reduction; the host gather
is a pure concatenation.

Per-core pipeline (all feature-major "T" layouts = [feature, seq]):
  - All matmul operands are float32r: 1 PE cycle/row at moving dim >= 256 vs
    4 for fp32. Every fp32r-consumed tensor is produced either by a DMA from
    an fp32r DRAM tensor or by a DVE/ACT op with fp32r output (walrus
    requires producers to round).
  - The key axis is host-rotated by each core's query offset (softmax and
    attn@v are permutation-invariant over keys when k/pos_k/v rotate
    together), which puts this core's pos_q rows at a fixed offset of the
    pos_k input — the pos tensor ships once instead of twice.
  - kcat/qcat: per head h, a 32-partition block [k_h (16 rows); pos_k_h (16)]
    (resp. [q_h * scale; pos_q_h]) so scoresT = kcat_blk^T @ qcat_blk fuses
    the qk and positional terms into ONE K=32 matmul per strip.
  - Main loop is paced by the Activation engine (exp): score tiles are
    [128 keys, 2 heads x 512 q] = 2 PSUM banks, emitted 3 tiles ahead (6
    banks); each exp is one 1024-column ACT instruction (the ~185 ns
    per-instruction ACT overhead argues for wide tiles; 2048-wide needs more
    PSUM than the accumulators leave). attn@v accumulates over the 16 key
    chunks directly in PSUM via start/stop flags in a 2-bank pool shared (by
    allocation-order rotation) with the v-projection and output-projection
    scratch, so nothing downstream ever blocks the score->exp chain.
  - v is augmented to 32 columns per head [1 | v_h | 0*15]: attn@v, the
    softmax row-sums, and zeros for the padding rows come from one matmul.
  - normalization (divide by the row-sum broadcast via stream_shuffle)
    happens once per (q-block, head-group) on the idle DVE; Wo is
    host-permuted to read the scattered [head-block @ 32j] layout directly.
"""

import numpy as np

H, DK, DM = 8, 16, 128
B, S = 4, 2048
R = 1024  # query rows per core
NCORES = 8
NKC = S // 128  # 16 key chunks
QB = 512  # q block
NQB = R // QB  # 2
LOOK = 3  # score-tile lookahead (PSUM banks: 2*LOOK + 2 shared acc/po/pv = 8)

_CACHE = {}


def _patch_drain():
    """walrus on this stack rejects >1 sync-wait on CTRL instructions; the
    TileContext exit drain can carry several. Absorb them on SP nops first."""
    import concourse.mybir as mybir
    from concourse.tile import TileContext, ScopedClock

    if getattr(TileContext, "_drain_patched", False):
        return
    orig = TileContext._drain_and_barrier

    def patched(self, tick_clock, wait_clock):
        nc = self.nc
        probe = nc.sync.nop(nofuse=True)
        wait_clock.add_sem_waits(
            probe.ins, ScopedClock({None: tick_clock.global_clock})
        )
        w = list(probe.ins.sync_info.on_wait or []) if probe.ins.sync_info else []
        if len(w) > 1:
            probe.ins.sync_info.on_wait = w[:1]
            for i in range(1, len(w)):
                n2 = nc.sync.nop(nofuse=True)
                n2.ins.sync_info = mybir.SyncInfo(on_wait=w[i : i + 1], on_update=[])

        class _NoWaits:
            def __init__(s, real):
                s._real = real

            def add_sem_waits(s, ins, clock):
                pass

            def __getattr__(s, k):
                return getattr(s._real, k)

        orig(self, tick_clock, _NoWaits(wait_clock))

    TileContext._drain_and_barrier = patched
    TileContext._drain_patched = True


def _split_multi_waits(nc, mybir):
    """walrus here accepts at most 1 sync-wait on most instruction structs
    (2 on EventSemaphore). Hoist excess waits onto same-engine NoOps placed
    immediately before the instruction — same blocking semantics."""
    for f in nc.m.functions:
        for blk in f.blocks:
            new_insts = []
            changed = False
            for inst in blk.instructions:
                si = inst.sync_info
                waits = list(si.on_wait) if si and si.on_wait else []
                limit = 2 if type(inst).__name__ == "InstEventSemaphore" else 1
                if len(waits) > limit:
                    changed = True
                    extra = waits[: len(waits) - limit]
                    for wv in extra:
                        n = mybir.InstNoOp(
                            name=f"wsplit_{nc.next_id()}",
                            engine=inst.engine,
                            ins=[],
                            outs=[],
                            sync_info=mybir.SyncInfo(on_wait=[wv], on_update=[]),
                        )
                        nc.register_instruction(n)
                        new_insts.append(n)
                    inst.sync_info.on_wait = waits[len(waits) - limit :]
                new_insts.append(inst)
            if changed:
                blk.instructions = new_insts


def build_bass(mm_dtype="float32r", presim=False):
    import concourse.bass as bass
    import concourse.mybir as mybir
    import concourse.tile as tile

    _patch_drain()
    dt = mybir.dt
    f32 = dt.float32
    mmdt = getattr(dt, mm_dtype)
    # fp32r matmuls require dst partition offset 0; the attn@v matmuls write
    # 32-row strips at 32j, so they run in bf16 instead (same 1 cycle/row,
    # and only the post-softmax weights + v go through it)
    avdt = dt.bfloat16 if mm_dtype == "float32r" else mmdt
    AF = mybir.ActivationFunctionType
    OP = mybir.AluOpType

    nc = bass.Bass("TRN2", num_devices=NCORES, enable_asserts=True)

    kp_d = nc.dram_tensor("kp", [DM, 2 * S], mmdt, kind="ExternalInput")
    qp_d = nc.dram_tensor("qp", [DM, R], mmdt, kind="ExternalInput")
    # v ships in bf16 (it only feeds the bf16 v_aug path; halves its DMA
    # time) with a bf16 W2^T appended — same dtype end to end, no bitcast
    xv_d = nc.dram_tensor("xv", [DM, S + DM], avdt, kind="ExternalInput")
    # weights + 4 bias columns + the 256-col v_aug template in one load
    # (the f32 extras are bitcast back out of the fp32r pack)
    wp_d = nc.dram_tensor("wp", [DM, 6 * DM + 4 + 32 * H], mmdt, kind="ExternalInput")
    outT_d = nc.dram_tensor("outT", [DM, R], f32, kind="ExternalOutput")

    with tile.TileContext(nc) as tc:
        with (
            tc.tile_pool(name="singles", bufs=1) as singles,
            tc.tile_pool(name="exps", bufs=4) as exps,
            tc.tile_pool(name="tailp", bufs=2) as tailp,
        ):
            def loadt(name, shape, dtype):
                return singles.tile(shape, dtype, tag=name, name=name)

            # loads in need-order: weights+biases (tiny), v template, keys,
            # pos (kp second half), q, v
            s_w = loadt("w_s", [DM, 6 * DM + 4 + 32 * H], mmdt)
            s_kp = loadt("kp_s", [DM, 2 * S], mmdt)
            s_qp = loadt("qp_s", [DM, R], mmdt)
            s_xv = loadt("xv_s", [DM, S + DM], avdt)
            # per head: [1 | v_h (16) | 0*15] -> av matmul also emits row sums
            # (ones col) and hard zeros in the padding rows of each 32-block
            v_aug = singles.tile([DM, NKC, 32 * H], avdt, tag="vaug", name="vaug")
            va4 = v_aug.rearrange("p t (h u) -> p t h u", u=32)
            nc.sync.dma_start(out=s_w[:, :], in_=wp_d[:, :])
            for c in range(4):
                nc.sync.dma_start(
                    out=s_kp[:, c * 1024 : (c + 1) * 1024],
                    in_=kp_d[:, c * 1024 : (c + 1) * 1024],
                )
            nc.sync.dma_start(out=s_qp[:, 0:QB], in_=qp_d[:, 0:QB])
            nc.sync.dma_start(out=s_qp[:, QB:R], in_=qp_d[:, QB:R])
            nc.sync.dma_start(out=s_xv[:, :], in_=xv_d[:, :])

            w1T = s_w[:, 0:128]
            w0Ts = s_w[:, 128:256]
            w0T = s_w[:, 256:384]
            w2T = s_w[:, 384:512]
            woPA = s_w[:, 512:640]
            woPB = s_w[:, 640:768]
            s_bias = s_w[:, 768:772].bitcast(f32)
            b1c = s_bias[:, 0:1]
            b0s = s_bias[:, 1:2]
            b0c = s_bias[:, 2:3]
            boc = s_bias[:, 3:4]
            # v_aug template [1 | b2_h (16) | 0*15] per head block
            vtpl = s_w[:, 772 : 772 + 32 * H].bitcast(f32)

            kcat = [
                singles.tile([DM, S], mmdt, tag="kcatA", name="kcatA"),
                singles.tile([DM, S], mmdt, tag="kcatB", name="kcatB"),
            ]
            qcat = [
                singles.tile([DM, R], mmdt, tag="qcatA", name="qcatA"),
                singles.tile([DM, R], mmdt, tag="qcatB", name="qcatB"),
            ]

            xs = [
                singles.tile([DM, R], mmdt, tag="xsA", name="xsA"),
                singles.tile([DM, R], mmdt, tag="xsB", name="xsB"),
            ]
            ob = singles.tile([DM, R], f32, tag="ob", name="ob")

            # ---------------- projections ----------------
            kT_sb = singles.tile([DM, S], mmdt, tag="kT_sb", name="kT_sb")
            pkT_sb = singles.tile([DM, S], mmdt, tag="pkT_sb", name="pkT_sb")
            qT_sb = singles.tile([DM, R], mmdt, tag="qT_sb", name="qT_sb")
            pqT_sb = singles.tile([DM, R], mmdt, tag="pqT_sb", name="pqT_sb")

            evac_flip = [0]

            def evac(dst_ap, src_ap, bias):
                # alternate evac between DVE and ACT so neither serializes
                if evac_flip[0] % 2 == 0:
                    nc.vector.tensor_scalar_add(out=dst_ap, in0=src_ap, scalar1=bias)
                else:
                    nc.scalar.activation(
                        out=dst_ap, in_=src_ap, func=AF.Identity, bias=bias
                    )
                evac_flip[0] += 1

            def interleave(g, half, src_sb, dst, c0=0, c1=None):
                # parity interleave: dst[g][2d + half] = src[64g + d]; a head
                # block is dst rows 32j..32j+32 (k/pos rows alternating — the
                # K=32 contraction is order-invariant). One DMA per column
                # range, and its out partition pattern is a single stride-2
                # dim (multi-level partition patterns are not expressible in
                # one AP). g0's interleaves are column-chunked so the first
                # score tiles start before the full tensors land (key chunks
                # are consumed sequentially, so the rest arrives mid-loop).
                if c1 is None:
                    c1 = dst.shape[-1]
                dst_v = dst.rearrange("(d two) c -> d two c", two=2)[:, half, c0:c1]
                src_v = src_sb[64 * g : 64 * g + 64, c0:c1]
                nc.sync.dma_start(out=dst_v, in_=src_v)

            with tc.tile_pool(name="proj_psum", bufs=4, space="PSUM") as proj_psum:
                # PE p-state warmup: ~3us of continuous dummy work brings the
                # PE to full clock before the real projections land
                warm = proj_psum.tile([128, 512], f32, tag="warm", name="warm")
                for _ in range(4):
                    nc.tensor.matmul(
                        out=warm[:, :],
                        lhsT=w1T,
                        rhs=s_w[:, 0:512],
                        start=True,
                        stop=True,
                    )

                def proj(lhsT, rhs_src, ncols, bias, dst_sb):
                    for c0 in range(0, ncols, 512):
                        pk = proj_psum.tile([128, 512], f32, tag="proj", name="pk")
                        nc.tensor.matmul(
                            out=pk[:, :],
                            lhsT=lhsT,
                            rhs=rhs_src[:, c0 : c0 + 512],
                            start=True,
                            stop=True,
                        )
                        evac(dst_sb[:, c0 : c0 + 512], pk[:, :], bias)

                proj(w1T, s_kp[:, 0:S], S, b1c, kT_sb)
                proj(w1T, s_kp[:, S : 2 * S], S, b1c, pkT_sb)
                # g0's kcat/qcat gate the first score tile; g1's interleaves
                # and the later column chunks run during the loop
                interleave(0, 0, kT_sb, kcat[0], 0, 1024)
                interleave(0, 1, pkT_sb, kcat[0], 0, 1024)
                proj(w0Ts, s_qp, R, b0s, qT_sb)
                # pos_q rows sit at kp[:, S:S+R] thanks to the host key-roll
                proj(w0T, s_kp[:, S : S + R], R, b0c, pqT_sb)
                interleave(0, 0, qT_sb, qcat[0], 0, QB)
                interleave(0, 1, pqT_sb, qcat[0], 0, QB)
                interleave(0, 0, kT_sb, kcat[0], 1024, 2048)
                interleave(0, 1, pkT_sb, kcat[0], 1024, 2048)
                interleave(0, 0, qT_sb, qcat[0], QB, R)
                interleave(0, 1, pqT_sb, qcat[0], QB, R)
                interleave(1, 0, kT_sb, kcat[1])
                interleave(1, 1, pkT_sb, kcat[1])
                interleave(1, 0, qT_sb, qcat[1])
                interleave(1, 1, pqT_sb, qcat[1])

            # ---------------- attention main loop ----------------
            # iter = (qc, g, kc, jj): head pair jj of group g, key chunk kc,
            # q block qc. kc outer of jj so each acc strip sees kc in order.
            ITERS = [
                (qc, g, kc, jj)
                for qc in range(NQB)
                for g in (0, 1)
                for kc in range(NKC)
                for jj in (0, 1)
            ]
            T = len(ITERS)
            sct = {}
            accs = {}
            pos = {}

            # acc/po/pv share one 2-bank pool (same tag): the allocation
            # sequence vg0..vg3, acc(0,0), po(0), acc(0,1), acc(1,0), po(1),
            # acc(1,1) alternates buffers so each grab lands on a freed bank
            with (
                tc.tile_pool(name="sc_psum", bufs=LOOK, space="PSUM") as sc_psum,
                tc.tile_pool(name="accpo_psum", bufs=2, space="PSUM") as accpo_psum,
            ):
                # stamp the [1 | b2 | 0*15] template on the idle Pool engine
                # (only needs the weight pack, so all 16 run right away and
                # never pace the v-projection chain below)
                for t in range(NKC):
                    nc.gpsimd.tensor_scalar_add(
                        out=v_aug[:, t, :], in0=vtpl, scalar1=0.0
                    )

                def emit_vgroup(g4):
                    # v projection, seq-major, 4 key-chunks batched into one
                    # PSUM bank (start/stop group flags give replace
                    # semantics per 128-col block) and evacuated by a single
                    # in-place add over the pre-stamped template — the 4
                    # matmuls outlast the one TT add, so the chain streams
                    pv = accpo_psum.tile([128, QB], f32, tag="accpo", name="pv")
                    for c in range(4):
                        t = 4 * g4 + c
                        nc.tensor.matmul(
                            out=pv[:, 128 * c : 128 * (c + 1)],
                            lhsT=s_xv[:, t * 128 : (t + 1) * 128],
                            rhs=s_xv[:, S : S + DM],
                            start=(c == 0),
                            stop=(c == 3),
                        )
                    nc.vector.tensor_tensor(
                        out=va4[:, 4 * g4 : 4 * g4 + 4, :, 1:17],
                        in0=pv.rearrange("p (t h u) -> p t h u", t=4, u=16),
                        in1=va4[:, 4 * g4 : 4 * g4 + 4, :, 1:17],
                        op=OP.add,
                    )

                def emit_sc(t):
                    qc, g, kc, jj = ITERS[t]
                    st = sc_psum.tile([128, 2 * QB], f32, tag="sc", name="sc")
                    sct[t] = st
                    for i in (0, 1):
                        j = 2 * jj + i
                        nc.tensor.matmul(
                            out=st[:, QB * i : QB * (i + 1)],
                            lhsT=kcat[g][
                                32 * j : 32 * j + 32, kc * 128 : (kc + 1) * 128
                            ],
                            rhs=qcat[g][32 * j : 32 * j + 32, qc * QB : (qc + 1) * QB],
                            start=True,
                            stop=True,
                            tile_position=(32 * j, 0),
                        )

                for g4 in range(4):
                    emit_vgroup(g4)
                for t in range(LOOK):
                    emit_sc(t)
                for t in range(T):
                    qc, g, kc, jj = ITERS[t]
                    if kc == 0 and jj == 0:
                        accs[(qc, g)] = accpo_psum.tile(
                            [128, QB], f32, tag="accpo", name="acc"
                        )
                    acc = accs[(qc, g)]
                    e = exps.tile([128, 2 * QB], avdt, tag="e", name="e")
                    nc.scalar.activation(out=e[:, :], in_=sct[t][:, :], func=AF.Exp)
                    for i in (0, 1):
                        j = 2 * jj + i
                        h = 4 * g + j
                        nc.tensor.matmul(
                            out=acc[32 * j : 32 * j + 32, :],
                            lhsT=v_aug[:, kc, 32 * h : 32 * h + 32],
                            rhs=e[:, QB * i : QB * (i + 1)],
                            start=(kc == 0),
                            stop=(kc == NKC - 1),
                            tile_position=(0, 32 * j),
                            # CoreSim's group-started bookkeeping mismaps
                            # partition-offset outputs (its pending-zero
                            # value model is correct); the 4 strip groups
                            # here are partition-disjoint by construction
                            skip_group_check=True,
                        )
                    del sct[t]
                    if t + LOOK < T:
                        emit_sc(t + LOOK)

                    if kc == NKC - 1 and jj == 1:
                        # (qc, g) head-group done: normalize into xs[g]
                        qsl = slice(qc * QB, (qc + 1) * QB)
                        sr = tailp.tile([DM, QB], f32, tag="sr", name="sr")
                        nc.vector.stream_shuffle(
                            out=sr[:, :], in_=acc[:, :], mask=[0] * 32
                        )
                        rc = tailp.tile([DM, QB], f32, tag="rc", name="rc")
                        nc.vector.reciprocal(out=rc[:, :], in_=sr[:, :])
                        nc.vector.tensor_tensor(
                            out=xs[g][:, qsl],
                            in0=acc[:, :],
                            in1=rc[:, :],
                            op=OP.mult,
                        )
                        del accs[(qc, g)]
                        if g == 0:
                            # start the output projection on the first half
                            pos[qc] = accpo_psum.tile(
                                [128, QB], f32, tag="accpo", name="po"
                            )
                            nc.tensor.matmul(
                                out=pos[qc][:, :],
                                lhsT=woPA,
                                rhs=xs[0][:, qsl],
                                start=True,
                                stop=False,
                            )
                        else:
                            po = pos.pop(qc)
                            nc.tensor.matmul(
                                out=po[:, :],
                                lhsT=woPB,
                                rhs=xs[1][:, qsl],
                                start=False,
                                stop=True,
                            )
                            for v2 in (0, 1):
                                osl = slice(
                                    qc * QB + v2 * (QB // 2),
                                    qc * QB + (v2 + 1) * (QB // 2),
                                )
                                nc.vector.tensor_scalar_add(
                                    out=ob[:, osl],
                                    in0=po[:, v2 * (QB // 2) : (v2 + 1) * (QB // 2)],
                                    scalar1=boc,
                                )
                                nc.sync.dma_start(
                                    out=outT_d[:, osl], in_=ob[:, osl]
                                )


    _split_multi_waits(nc, mybir)
    return nc


def shard_inputs(query, key, value, pos_embed, W0, b0, W1, b1, W2, b2, Wo, bo):
    """Build the 8 per-core input maps (host-side layout preprocessing)."""
    f = np.float32
    asc = np.ascontiguousarray
    cat = np.concatenate
    scale = 1.0 / np.sqrt(np.float32(DK))

    woPA = np.zeros((DM, DM), f)
    woPB = np.zeros((DM, DM), f)
    WoT = asc(Wo.T.astype(f))
    for j in range(4):
        woPA[32 * j + 1 : 32 * j + 17, :] = WoT[16 * j : 16 * j + 16, :]
        woPB[32 * j + 1 : 32 * j + 17, :] = WoT[16 * (4 + j) : 16 * (4 + j) + 16, :]

    import ml_dtypes

    bf16 = ml_dtypes.bfloat16
    vtpl = np.zeros((DM, 32 * H), f)
    for h in range(H):
        vtpl[:, 32 * h] = 1.0
        vtpl[:, 32 * h + 1 : 32 * h + 17] = b2.astype(f)[None, 16 * h : 16 * h + 16]
    wp = asc(
        cat(
            [
                W1.T.astype(f),
                (W0.T * scale).astype(f),
                W0.T.astype(f),
                W2.T.astype(f),
                woPA,
                woPB,
                b1.astype(f).reshape(DM, 1),
                (b0 * scale).astype(f).reshape(DM, 1),
                b0.astype(f).reshape(DM, 1),
                bo.astype(f).reshape(DM, 1),
                vtpl,
            ],
            axis=1,
        )
    )
    shared = {"wp": wp}
    in_maps = []
    for c in range(NCORES):
        b_i, half = divmod(c, 2)
        r0 = half * R
        # rotate the key axis by r0 (k/pos_k/v together — softmax and attn@v
        # are permutation-invariant over keys) so pos_q = pos rows r0..r0+R
        # sits at kp[:, S:S+R]
        perm = np.roll(np.arange(S), -r0)
        in_maps.append(
            dict(
                shared,
                kp=asc(
                    cat(
                        [key[b_i][perm].T, pos_embed[b_i][perm].T], axis=1
                    ).astype(f)
                ),
                qp=asc(query[b_i, r0 : r0 + R, :].T.astype(f)),
                xv=asc(cat([value[b_i][perm].T, W2.T], axis=1).astype(f).astype(bf16)),
            )
        )
    return in_maps


def gather_outputs(results):
    out = np.empty((B, S, DM), np.float32)
    for c in range(NCORES):
        b_i, half = divmod(c, 2)
        r0 = half * R
        out[b_i, r0 : r0 + R, :] = results[c]["outT"].T
    return out


def kernel(query, key, value, pos_embed, W0, b0, W1, b1, W2, b2, Wo, bo):
    from concourse.bass_utils import run_bass_kernel_spmd

    # inputs may arrive as jax arrays; materialize once so the host-side
    # slicing/transposing below stays in numpy
    args = [
        np.asarray(a)
        for a in (query, key, value, pos_embed, W0, b0, W1, b1, W2, b2, Wo, bo)
    ]
    if "nc" not in _CACHE:
        _CACHE["nc"] = build_bass()
    in_maps = shard_inputs(*args)
    res = run_bass_kernel_spmd(_CACHE["nc"], in_maps, core_ids=list(range(NCORES)))
    return gather_outputs(res.results)



# revision 12
# speedup vs baseline: 1.4624x; 1.4624x over previous
"""Multi-head attention with additive positional attention — TRN2 Bass kernel.

Problem: B=4, S=2048, DM=128, H=8, DK=16.
  scores = (q @ k^T) / sqrt(DK) + pos_q @ pos_k^T   per (b, h)
  out    = softmax(scores) @ v, heads merged, @ Wo^T + bo

Sharding: 8 cores = batch (4) x query-row halves (2). Each core holds one
batch's full keys/values (S=2048) and 1024 query rows, computes all 8 heads,
and produces complete output rows — no cross-core reduction; the host gather
is a pure concatenation.

Per-core pipeline (all matmul operands fp32r or bf16; feature-major "T"
layouts = [feature, seq]):
  - kcat/qcat: per head h, a 32-partition block [k_h; pos_k_h] (resp.
    [q_h * scale; pos_q_h]) interleaved so scoresT = kcat_blk^T @ qcat_blk
    fuses the qk and positional terms into ONE K=32 matmul per strip. The
    key axis is host-rotated per core so pos_q rows ship inside the pos
    tensor (see shard_inputs).
  - The exp is the throughput limit of the whole kernel (131072 columns of
    128 lanes). It is SPLIT across the Activation engine (native Exp) and
    the DVE (Schraudolph bit-trick: e ~= bitcast_bf16(int16(s*128*log2e +
    16256-C)) — one tensor_scalar mult+add with int16-converting output;
    HW rounds to nearest; ~1.8% rms multiplicative error on the affected
    tiles, ~0.5% end-to-end after softmax common-mode cancellation).
    GPSIMD/Pool cannot read PSUM, so it only stamps the v_aug template.
  - attn@v runs with the exp'd scores as the STATIONARY operand
    (lhsT = e [128 keys, 128 q]) and v as the moving one (rhs [128, 32]):
    out = [128 q-partitions, 32], accumulated over key chunks directly in
    PSUM. v_aug per head is [1*16 | v_h] so the same matmul emits the
    softmax row-sums replicated 16x — normalization becomes two plain DVE
    ops (reciprocal + multiply), no partition broadcast needed.
  - The q-major attention output is flipped back to feature-major with PE
    transposes (bf16, via identity), then the output projection is a single
    K=128 bf16 matmul per 512-query block.
  - PSUM: 3 score tiles [128,1024] (6 banks) + 1 shared acc/v-proj bank +
    1 shared transpose/out-proj bank = 8 banks exactly. Accumulator strips
    share one bank with a SINGLE start_tensor_calc on the first write
    (replace-semantics for every first touch of each byte range), as the
    v-projection has always done.
"""

import numpy as np

H, DK, DM = 8, 16, 128
B, S = 4, 2048
R = 1024  # query rows per core
NCORES = 8
NKC = S // 128  # 16 key chunks
QB = 512  # q block
NQB = R // QB  # 2
LOOK = 3  # score-tile lookahead (PSUM banks: 2*LOOK + acc 1 + misc 1 = 8)

# Schraudolph exp trick constants (int16 output bitcast as bf16):
#   e(s) = bitcast_bf16(round(s * 128*log2e + (16256 - C)))
EXP_A = 128.0 * 1.4426950408889634
EXP_C = 7.33
EXP_B = 16256.0 - EXP_C

# of the 128 score tiles, how many the ACT engine exps (rest: DVE trick)
N_ACT = 70

_CACHE = {}


def _patch_drain():
    """walrus on this stack rejects >1 sync-wait on CTRL instructions; the
    TileContext exit drain can carry several. Absorb them on SP nops first."""
    import concourse.mybir as mybir
    from concourse.tile import TileContext, ScopedClock

    if getattr(TileContext, "_drain_patched", False):
        return
    orig = TileContext._drain_and_barrier

    def patched(self, tick_clock, wait_clock):
        nc = self.nc
        probe = nc.sync.nop(nofuse=True)
        wait_clock.add_sem_waits(
            probe.ins, ScopedClock({None: tick_clock.global_clock})
        )
        w = list(probe.ins.sync_info.on_wait or []) if probe.ins.sync_info else []
        if len(w) > 1:
            probe.ins.sync_info.on_wait = w[:1]
            for i in range(1, len(w)):
                n2 = nc.sync.nop(nofuse=True)
                n2.ins.sync_info = mybir.SyncInfo(on_wait=w[i : i + 1], on_update=[])

        class _NoWaits:
            def __init__(s, real):
                s._real = real

            def add_sem_waits(s, ins, clock):
                pass

            def __getattr__(s, k):
                return getattr(s._real, k)

        orig(self, tick_clock, _NoWaits(wait_clock))

    TileContext._drain_and_barrier = patched
    TileContext._drain_patched = True


def _split_multi_waits(nc, mybir):
    """walrus here accepts at most 1 sync-wait on most instruction structs
    (2 on EventSemaphore). Hoist excess waits onto same-engine NoOps placed
    immediately before the instruction — same blocking semantics."""
    for f in nc.m.functions:
        for blk in f.blocks:
            new_insts = []
            changed = False
            for inst in blk.instructions:
                si = inst.sync_info
                waits = list(si.on_wait) if si and si.on_wait else []
                limit = 2 if type(inst).__name__ == "InstEventSemaphore" else 1
                if len(waits) > limit:
                    changed = True
                    extra = waits[: len(waits) - limit]
                    for wv in extra:
                        n = mybir.InstNoOp(
                            name=f"wsplit_{nc.next_id()}",
                            engine=inst.engine,
                            ins=[],
                            outs=[],
                            sync_info=mybir.SyncInfo(on_wait=[wv], on_update=[]),
                        )
                        nc.register_instruction(n)
                        new_insts.append(n)
                    inst.sync_info.on_wait = waits[len(waits) - limit :]
                new_insts.append(inst)
            if changed:
                blk.instructions = new_insts


def _exp_engine_pattern():
    """Weighted round-robin: N_ACT of the 128 tiles on ACT, rest on DVE."""
    pat = []
    accu = 0
    for _ in range(NQB * 2 * NKC * 2):
        accu += N_ACT
        if accu >= 128:
            accu -= 128
            pat.append("A")
        else:
            pat.append("D")
    return pat


def build_bass(mm_dtype="float32r", presim=False):
    import concourse.bass as bass
    import concourse.mybir as mybir
    import concourse.tile as tile

    _patch_drain()
    dt = mybir.dt
    f32 = dt.float32
    bf16 = dt.bfloat16
    i16 = dt.int16
    mmdt = bf16  # whole data path is bf16 (DMA bytes are the startup gate)
    AF = mybir.ActivationFunctionType
    OP = mybir.AluOpType

    nc = bass.Bass("TRN2", num_devices=NCORES, enable_asserts=True)

    kp_d = nc.dram_tensor("kp", [DM, 2 * S], mmdt, kind="ExternalInput")
    qp_d = nc.dram_tensor("qp", [DM, R], mmdt, kind="ExternalInput")
    # v ships in bf16 (it only feeds the bf16 v_aug path) with bf16 W2^T
    xv_d = nc.dram_tensor("xv", [DM, S + DM], bf16, kind="ExternalInput")
    # weights (w1T, w0Ts, w0T) + 4 f32 biases + v_aug template + WoT + identity
    WPW = 3 * DM + 8 + 32 * H + 2 * DM  # 904 bf16 columns
    wp_d = nc.dram_tensor("wp", [DM, WPW], mmdt, kind="ExternalInput")
    outT_d = nc.dram_tensor("outT", [DM, R], f32, kind="ExternalOutput")

    ENG = _exp_engine_pattern()

    with tile.TileContext(nc) as tc:
        with (
            tc.tile_pool(name="singles", bufs=1) as singles,
            tc.tile_pool(name="exps", bufs=8) as exps,
            tc.tile_pool(name="tailp", bufs=2) as tailp,
        ):
            def loadt(name, shape, dtype):
                return singles.tile(shape, dtype, tag=name, name=name)

            s_w = loadt("w_s", [DM, WPW], mmdt)
            s_kp = loadt("kp_s", [DM, 2 * S], mmdt)
            s_qp = loadt("qp_s", [DM, R], mmdt)
            s_xv = loadt("xv_s", [DM, S + DM], bf16)
            # per head: [1*16 | v_h] -> the attn@v matmul also emits row sums
            # replicated over the first 16 columns of each 32-block
            v_aug = singles.tile([DM, NKC, 32 * H], bf16, tag="vaug", name="vaug")
            va4 = v_aug.rearrange("p t (h u) -> p t h u", u=32)
            nc.sync.dma_start(out=s_w[:, :], in_=wp_d[:, :])
            for c in range(4):
                nc.sync.dma_start(
                    out=s_kp[:, c * 1024 : (c + 1) * 1024],
                    in_=kp_d[:, c * 1024 : (c + 1) * 1024],
                )
            nc.sync.dma_start(out=s_qp[:, 0:QB], in_=qp_d[:, 0:QB])
            nc.sync.dma_start(out=s_qp[:, QB:R], in_=qp_d[:, QB:R])
            nc.sync.dma_start(out=s_xv[:, :], in_=xv_d[:, :])

            w1T = s_w[:, 0:128]
            w0Ts = s_w[:, 128:256]
            w0T = s_w[:, 256:384]
            s_bias = s_w[:, 384:392].bitcast(f32)  # 4 f32 packed in 8 bf16 cols
            b1c = s_bias[:, 0:1]
            b0s = s_bias[:, 1:2]
            b0c = s_bias[:, 2:3]
            boc = s_bias[:, 3:4]
            # v_aug template [1*16 | b2_h] per head block
            vtpl = s_w[:, 392 : 392 + 32 * H]
            woT_b = s_w[:, 648:776]  # [128, 128] bf16 WoT (head-dense rows)
            identb = s_w[:, 776:904]  # [128, 128] bf16 identity

            kcat = [
                singles.tile([DM, S], mmdt, tag="kcatA", name="kcatA"),
                singles.tile([DM, S], mmdt, tag="kcatB", name="kcatB"),
            ]
            qcat = [
                singles.tile([DM, R], mmdt, tag="qcatA", name="qcatA"),
                singles.tile([DM, R], mmdt, tag="qcatB", name="qcatB"),
            ]

            ob = singles.tile([DM, R], f32, tag="ob", name="ob")

            # ---------------- projections ----------------
            kT_sb = singles.tile([DM, S], mmdt, tag="kT_sb", name="kT_sb")
            pkT_sb = singles.tile([DM, S], mmdt, tag="pkT_sb", name="pkT_sb")
            qT_sb = singles.tile([DM, R], mmdt, tag="qT_sb", name="qT_sb")
            pqT_sb = singles.tile([DM, R], mmdt, tag="pqT_sb", name="pqT_sb")

            evac_flip = [0]

            def evac(dst_ap, src_ap, bias):
                # alternate evac between DVE and ACT so neither serializes
                if evac_flip[0] % 2 == 0:
                    nc.vector.tensor_scalar_add(out=dst_ap, in0=src_ap, scalar1=bias)
                else:
                    nc.scalar.activation(
                        out=dst_ap, in_=src_ap, func=AF.Identity, bias=bias
                    )
                evac_flip[0] += 1

            def interleave(g, half, src_sb, dst, c0=0, c1=None):
                # parity interleave: dst[g][2d + half] = src[64g + d]; a head
                # block is dst rows 32j..32j+32 (k/pos rows alternating — the
                # K=32 contraction is order-invariant). g0's interleaves are
                # column-chunked so the first score tiles start early.
                if c1 is None:
                    c1 = dst.shape[-1]
                dst_v = dst.rearrange("(d two) c -> d two c", two=2)[:, half, c0:c1]
                src_v = src_sb[64 * g : 64 * g + 64, c0:c1]
                nc.sync.dma_start(out=dst_v, in_=src_v)

            # PE p-state warmup on a memset tile: starts immediately (no DMA
            # dependency) so the ramp overlaps the input transfers and the
            # projections land at full clock
            warm_sb = singles.tile([DM, 512], mmdt, tag="warm_sb", name="warm_sb")
            nc.gpsimd.memset(warm_sb[:, :], 1.0)

            with tc.tile_pool(name="proj_psum", bufs=4, space="PSUM") as proj_psum:
                warm = proj_psum.tile([128, 512], f32, tag="warm", name="warm")
                for _ in range(6):
                    nc.tensor.matmul(
                        out=warm[:, :],
                        lhsT=warm_sb[:, 0:128],
                        rhs=warm_sb[:, :],
                        start=True,
                        stop=True,
                    )

                def proj(lhsT, rhs_src, ncols, bias, dst_sb):
                    for c0 in range(0, ncols, 512):
                        pk = proj_psum.tile([128, 512], f32, tag="proj", name="pk")
                        nc.tensor.matmul(
                            out=pk[:, :],
                            lhsT=lhsT,
                            rhs=rhs_src[:, c0 : c0 + 512],
                            start=True,
                            stop=True,
                        )
                        evac(dst_sb[:, c0 : c0 + 512], pk[:, :], bias)

                proj(w1T, s_kp[:, 0:S], S, b1c, kT_sb)
                proj(w1T, s_kp[:, S : 2 * S], S, b1c, pkT_sb)
                # g0's kcat/qcat gate the first score tile; g1's interleaves
                # and the later column chunks run during the loop
                interleave(0, 0, kT_sb, kcat[0], 0, 1024)
                interleave(0, 1, pkT_sb, kcat[0], 0, 1024)
                proj(w0Ts, s_qp, R, b0s, qT_sb)
                # pos_q rows sit at kp[:, S:S+R] thanks to the host key-roll
                proj(w0T, s_kp[:, S : S + R], R, b0c, pqT_sb)
                interleave(0, 0, qT_sb, qcat[0], 0, QB)
                interleave(0, 1, pqT_sb, qcat[0], 0, QB)
                interleave(0, 0, kT_sb, kcat[0], 1024, 2048)
                interleave(0, 1, pkT_sb, kcat[0], 1024, 2048)
                interleave(0, 0, qT_sb, qcat[0], QB, R)
                interleave(0, 1, pqT_sb, qcat[0], QB, R)
                interleave(1, 0, kT_sb, kcat[1])
                interleave(1, 1, pkT_sb, kcat[1])
                interleave(1, 0, qT_sb, qcat[1])
                interleave(1, 1, pqT_sb, qcat[1])

            # ---------------- attention main loop ----------------
            # iter = (qc, g, kc, jj): head pair jj of group g, key chunk kc,
            # q block qc. kc outer of jj so each acc bank sees kc in order.
            ITERS = [
                (qc, g, kc, jj)
                for qc in range(NQB)
                for g in (0, 1)
                for kc in range(NKC)
                for jj in (0, 1)
            ]
            T = len(ITERS)
            sct = {}
            ets = {}
            accs = {}
            xss = {}
            misc = {}

            with (
                tc.tile_pool(name="sc_psum", bufs=LOOK, space="PSUM") as sc_psum,
                tc.tile_pool(name="acc_psum", bufs=1, space="PSUM") as acc_psum,
                tc.tile_pool(name="misc_psum", bufs=1, space="PSUM") as misc_psum,
            ):
                # stamp the [1*16 | b2] template on the idle Pool engine
                for t in range(NKC):
                    nc.gpsimd.tensor_scalar_add(
                        out=v_aug[:, t, :], in0=vtpl, scalar1=0.0
                    )

                def emit_vgroup(g4):
                    # v projection, seq-major, 4 key-chunks batched into one
                    # PSUM bank (single start_tensor_calc on the first write:
                    # replace semantics per first touch) and evacuated by a
                    # single in-place add over the pre-stamped template
                    pv = acc_psum.tile([128, QB], f32, tag="accv", name="pv")
                    for c in range(4):
                        t = 4 * g4 + c
                        nc.tensor.matmul(
                            out=pv[:, 128 * c : 128 * (c + 1)],
                            lhsT=s_xv[:, t * 128 : (t + 1) * 128],
                            rhs=s_xv[:, S : S + DM],
                            start=(c == 0),
                            stop=(c == 3),
                        )
                    nc.vector.tensor_tensor(
                        out=va4[:, 4 * g4 : 4 * g4 + 4, :, 16:32],
                        in0=pv.rearrange("p (t h u) -> p t h u", t=4, u=16),
                        in1=va4[:, 4 * g4 : 4 * g4 + 4, :, 16:32],
                        op=OP.add,
                    )

                def emit_sc(t):
                    qc, g, kc, jj = ITERS[t]
                    st = sc_psum.tile([128, 2 * QB], f32, tag="sc", name="sc")
                    sct[t] = st
                    for i in (0, 1):
                        j = 2 * jj + i
                        nc.tensor.matmul(
                            out=st[:, QB * i : QB * (i + 1)],
                            lhsT=kcat[g][
                                32 * j : 32 * j + 32, kc * 128 : (kc + 1) * 128
                            ],
                            rhs=qcat[g][32 * j : 32 * j + 32, qc * QB : (qc + 1) * QB],
                            start=True,
                            stop=True,
                            tile_position=(32 * j, 0),
                        )

                def emit_exp(t):
                    e = exps.tile([128, 2 * QB], bf16, tag="e", name="e")
                    ets[t] = e
                    if ENG[t] == "A":
                        nc.scalar.activation(
                            out=e[:, :], in_=sct[t][:, :], func=AF.Exp
                        )
                    else:
                        nc.vector.tensor_scalar(
                            out=e.bitcast(i16)[:, :],
                            in0=sct[t][:, :],
                            scalar1=EXP_A,
                            scalar2=EXP_B,
                            op0=OP.mult,
                            op1=OP.add,
                        )
                    del sct[t]

                def emit_attnv(t):
                    # attn@v: e stationary, v_aug moving. Runs ATT_D tiles
                    # behind the exp stream so a stalled accumulator (waiting
                    # on the previous group's normalize) never blocks the
                    # score-tile refill in the in-order PE stream.
                    qc, g, kc, jj = ITERS[t]
                    if kc == 0 and jj == 0:
                        accs[(qc, g)] = acc_psum.tile(
                            [128, QB], f32, tag="accv", name="acc"
                        )
                    acc4 = accs[(qc, g)].rearrange(
                        "p (qch j u) -> p qch j u", qch=4, u=32
                    )
                    e = ets.pop(t)
                    for i in (0, 1):
                        j = 2 * jj + i
                        h = 4 * g + j
                        for qch in range(4):
                            nc.tensor.matmul(
                                out=acc4[:, qch, j, :],
                                lhsT=e[:, QB * i + 128 * qch : QB * i + 128 * (qch + 1)],
                                rhs=v_aug[:, kc, 32 * h : 32 * h + 32],
                                start=(kc == 0 and jj == 0 and i == 0 and qch == 0),
                                stop=(kc == NKC - 1 and jj == 1 and i == 1 and qch == 3),
                                skip_group_check=True,
                            )
                    if kc == NKC - 1 and jj == 1:
                        emit_tail(qc, g, acc4)

                def emit_tail(qc, g, acc4):
                    # ---- (qc, g) done: normalize, transpose ----
                    rc = tailp.tile([DM, 256], f32, tag="rc", name="rc")
                    rc4 = rc.rearrange("p (qch j d) -> p qch j d", qch=4, d=16)
                    nc.vector.reciprocal(
                        out=rc4[:, :, :, :], in_=acc4[:, :, :, 0:16]
                    )
                    xs = tailp.tile([DM, 256], bf16, tag="xs", name="xs")
                    xs4 = xs.rearrange("p (qch j d) -> p qch j d", qch=4, d=16)
                    nc.vector.tensor_tensor(
                        out=xs4[:, :, :, :],
                        in0=acc4[:, :, :, 16:32],
                        in1=rc4[:, :, :, :],
                        op=OP.mult,
                    )
                    del accs[(qc, g)]

                    if g == 0:
                        misc[qc] = misc_psum.tile(
                            [128, QB], f32, tag="misc", name="pp"
                        )
                    pp = misc[qc]
                    ptr = pp.bitcast(bf16)  # [128, 1024] bf16 view
                    for qch in range(4):
                        nc.tensor.matmul(
                            out=ptr[64 * g : 64 * g + 64, 128 * qch : 128 * (qch + 1)],
                            lhsT=xs4[:, qch, :, :],
                            rhs=identb,
                            is_transpose=True,
                            start=(qch == 0),
                            stop=(qch == 3),
                            tile_position=(0, 64 * g),
                            skip_group_check=True,
                        )

                    if g == 1:
                        # ---- output projection, pipelined in halves after
                        # one full evac (po reuses pp's bank: its first write
                        # must come after ALL ptr reads in engine order) ----
                        xsT = tailp.tile([DM, QB], bf16, tag="xsT", name="xsT")
                        nc.scalar.activation(
                            out=xsT[:, :], in_=ptr[:, 0:QB], func=AF.Copy
                        )
                        po = misc_psum.tile([128, QB], f32, tag="misc", name="po")
                        del misc[qc]
                        for v2 in (0, 1):
                            hsl = slice(v2 * (QB // 2), (v2 + 1) * (QB // 2))
                            osl = slice(
                                qc * QB + v2 * (QB // 2),
                                qc * QB + (v2 + 1) * (QB // 2),
                            )
                            nc.tensor.matmul(
                                out=po[:, hsl],
                                lhsT=woT_b,
                                rhs=xsT[:, hsl],
                                start=(v2 == 0),
                                stop=(v2 == 1),
                                skip_group_check=True,
                            )
                            nc.scalar.activation(
                                out=ob[:, osl], in_=po[:, hsl],
                                func=AF.Identity, bias=boc,
                            )
                            nc.sync.dma_start(out=outT_d[:, osl], in_=ob[:, osl])

                ATT_D = 6  # attn@v emission delay (tiles)
                for g4 in range(4):
                    emit_vgroup(g4)
                for t in range(LOOK):
                    emit_sc(t)
                for t in range(T):
                    emit_exp(t)
                    if t >= ATT_D:
                        emit_attnv(t - ATT_D)
                    if t + LOOK < T:
                        emit_sc(t + LOOK)
                for t in range(T - ATT_D, T):
                    emit_attnv(t)

    _split_multi_waits(nc, mybir)
    return nc


def shard_inputs(query, key, value, pos_embed, W0, b0, W1, b1, W2, b2, Wo, bo):
    """Build the 8 per-core input maps (host-side layout preprocessing)."""
    f = np.float32
    asc = np.ascontiguousarray
    cat = np.concatenate
    scale = 1.0 / np.sqrt(np.float32(DK))

    import ml_dtypes

    bf16 = ml_dtypes.bfloat16

    # 4 f32 biases packed into 8 bf16 bit-container columns
    bias4 = asc(
        cat(
            [
                b1.astype(f).reshape(DM, 1),
                (b0 * scale).astype(f).reshape(DM, 1),
                b0.astype(f).reshape(DM, 1),
                bo.astype(f).reshape(DM, 1),
            ],
            axis=1,
        )
    ).view(np.uint16).view(bf16)

    # v_aug template: [ones(16) | b2_h(16)] per head block
    vtpl = np.zeros((DM, 32 * H), f)
    for h in range(H):
        vtpl[:, 32 * h : 32 * h + 16] = 1.0
        vtpl[:, 32 * h + 16 : 32 * h + 32] = b2.astype(f)[None, 16 * h : 16 * h + 16]

    # WoT in head-dense row order (hd = h*16+d) — natural Wo.T
    woT = np.asarray(Wo).T.astype(f)
    ident = np.eye(DM, dtype=f)

    wp = asc(
        np.concatenate(
            [
                W1.T.astype(f).astype(bf16),
                (W0.T * scale).astype(f).astype(bf16),
                W0.T.astype(f).astype(bf16),
                bias4,
                vtpl.astype(bf16),
                woT.astype(bf16),
                ident.astype(bf16),
            ],
            axis=1,
        )
    )
    shared = {"wp": wp}
    in_maps = []
    for c in range(NCORES):
        b_i, half = divmod(c, 2)
        r0 = half * R
        # rotate the key axis by r0 (k/pos_k/v together — softmax and attn@v
        # are permutation-invariant over keys) so pos_q = pos rows r0..r0+R
        # sits at kp[:, S:S+R]
        perm = np.roll(np.arange(S), -r0)
        in_maps.append(
            dict(
                shared,
                kp=asc(
                    cat(
                        [key[b_i][perm].T, pos_embed[b_i][perm].T], axis=1
                    ).astype(f).astype(bf16)
                ),
                qp=asc(query[b_i, r0 : r0 + R, :].T.astype(f).astype(bf16)),
                xv=asc(cat([value[b_i][perm].T, W2.T], axis=1).astype(f).astype(bf16)),
            )
        )
    return in_maps


def gather_outputs(results):
    out = np.empty((B, S, DM), np.float32)
    for c in range(NCORES):
        b_i, half = divmod(c, 2)
        r0 = half * R
        out[b_i, r0 : r0 + R, :] = results[c]["outT"].T
    return out


def kernel(query, key, value, pos_embed, W0, b0, W1, b1, W2, b2, Wo, bo):
    from concourse.bass_utils import run_bass_kernel_spmd

    # inputs may arrive as jax arrays; materialize once so the host-side
    # slicing/transposing below stays in numpy
    args = [
        np.asarray(a)
        for a in (query, key, value, pos_embed, W0, b0, W1, b1, W2, b2, Wo, bo)
    ]
    if "nc" not in _CACHE:
        _CACHE["nc"] = build_bass()
    in_maps = shard_inputs(*args)
    res = run_bass_kernel_spmd(_CACHE["nc"], in_maps, core_ids=list(range(NCORES)))
    return gather_outputs(res.results)


# revision 57
# speedup vs baseline: 1.5748x; 1.0768x over previous
"""Multi-head attention with additive positional attention — TRN2 Bass kernel.

Problem: B=4, S=2048, DM=128, H=8, DK=16.
  scores = (q @ k^T) / sqrt(DK) + pos_q @ pos_k^T   per (b, h)
  out    = softmax(scores) @ v, heads merged, @ Wo^T + bo

Sharding: 8 cores = batch (4) x query-row halves (2). Each core holds one
batch's full keys/values (S=2048) and 1024 query rows, computes all 8 heads,
and produces complete output rows — no cross-core reduction; the host gather
is a pure concatenation.

Per-core pipeline (whole data path bf16 — the sim charges all DMA traffic
to one serial 360 GB/s channel, so input/interleave bytes gate the startup;
feature-major "T" layouts = [feature, seq]):
  - kcat/qcat: per head h, a 32-partition block [k_h; pos_k_h] (resp.
    [q_h * scale; pos_q_h]) interleaved so scoresT = kcat_blk^T @ qcat_blk
    fuses the qk and positional terms into ONE K=32 matmul per strip. The
    key axis is host-rotated per core so pos_q rows ship inside the pos
    tensor (see shard_inputs).
  - The exp is the throughput limit of the whole kernel (131072 columns of
    128 lanes). It is SPLIT across the Activation engine (native Exp) and
    the DVE (Schraudolph bit-trick: e ~= bitcast_bf16(int16(s*128*log2e +
    16256-C)) — one tensor_scalar mult+add with int16-converting output;
    HW rounds to nearest; ~1.8% rms multiplicative error on the affected
    tiles, ~0.5% end-to-end after softmax common-mode cancellation).
    GPSIMD/Pool cannot read PSUM, so it only stamps the v_aug template.
  - attn@v runs with the exp'd scores as the STATIONARY operand
    (lhsT = e [128 keys, 128 q]) and v as the moving one (rhs [128, 17]):
    out = [128 q-partitions, 17], accumulated over key chunks directly in
    PSUM. v_aug per head is [1 | v_h] so the same matmul emits the softmax
    row-sums — normalization is one tiny DVE reciprocal plus one multiply
    with a stride-0 broadcast AP, no partition broadcast needed. attn@v
    emission lags the exp stream by ATT_D tiles so accumulator-bank reuse
    (waiting on the previous group's normalize) never blocks the
    score-tile refill in the in-order PE stream.
  - The q-major attention output is flipped back to feature-major with PE
    transposes (bf16, via identity), then the output projection is a single
    K=128 bf16 matmul per 512-query block.
  - PSUM: 3 score tiles [128,1024] (6 banks) + 1 shared acc/v-proj bank +
    1 shared transpose/out-proj bank = 8 banks exactly. Accumulator strips
    share one bank with a SINGLE start_tensor_calc on the first write
    (replace-semantics for every first touch of each byte range), as the
    v-projection has always done.
"""

import numpy as np

H, DK, DM = 8, 16, 128
B, S = 4, 2048
R = 1024  # query rows per core
NCORES = 8
NKC = S // 128  # 16 key chunks
QB = 512  # q block
NQB = R // QB  # 2
LOOK = 3  # score-tile lookahead (PSUM banks: 2*LOOK + acc 1 + misc 1 = 8)

# Schraudolph exp trick constants (int16 output bitcast as bf16):
#   e(s) = bitcast_bf16(round(s * 128*log2e + (16256 - C)))
EXP_A = 128.0 * 1.4426950408889634
EXP_C = 7.33
EXP_B = 16256.0 - EXP_C

_CACHE = {}


def _patch_drain():
    """walrus on this stack rejects >1 sync-wait on CTRL instructions; the
    TileContext exit drain can carry several. Absorb them on SP nops first."""
    import concourse.mybir as mybir
    from concourse.tile import TileContext, ScopedClock

    if getattr(TileContext, "_drain_patched", False):
        return
    orig = TileContext._drain_and_barrier

    def patched(self, tick_clock, wait_clock):
        nc = self.nc
        probe = nc.sync.nop(nofuse=True)
        wait_clock.add_sem_waits(
            probe.ins, ScopedClock({None: tick_clock.global_clock})
        )
        w = list(probe.ins.sync_info.on_wait or []) if probe.ins.sync_info else []
        if len(w) > 1:
            probe.ins.sync_info.on_wait = w[:1]
            for i in range(1, len(w)):
                n2 = nc.sync.nop(nofuse=True)
                n2.ins.sync_info = mybir.SyncInfo(on_wait=w[i : i + 1], on_update=[])

        class _NoWaits:
            def __init__(s, real):
                s._real = real

            def add_sem_waits(s, ins, clock):
                pass

            def __getattr__(s, k):
                return getattr(s._real, k)

        orig(self, tick_clock, _NoWaits(wait_clock))

    TileContext._drain_and_barrier = patched
    TileContext._drain_patched = True


def _split_multi_waits(nc, mybir):
    """walrus here accepts at most 1 sync-wait on most instruction structs
    (2 on EventSemaphore). Hoist excess waits onto same-engine NoOps placed
    immediately before the instruction — same blocking semantics."""
    for f in nc.m.functions:
        for blk in f.blocks:
            new_insts = []
            changed = False
            for inst in blk.instructions:
                si = inst.sync_info
                waits = list(si.on_wait) if si and si.on_wait else []
                limit = 2 if type(inst).__name__ == "InstEventSemaphore" else 1
                if len(waits) > limit:
                    changed = True
                    extra = waits[: len(waits) - limit]
                    for wv in extra:
                        n = mybir.InstNoOp(
                            name=f"wsplit_{nc.next_id()}",
                            engine=inst.engine,
                            ins=[],
                            outs=[],
                            sync_info=mybir.SyncInfo(on_wait=[wv], on_update=[]),
                        )
                        nc.register_instruction(n)
                        new_insts.append(n)
                    inst.sync_info.on_wait = waits[len(waits) - limit :]
                new_insts.append(inst)
            if changed:
                blk.instructions = new_insts


def _exp_engine_pattern():
    """Strict ACT/DVE alternation keeps the score-ring refill phase-locked;
    empirically better than busy-balancing toward ACT."""
    pat = ["A" if t % 2 == 0 else "D" for t in range(NQB * 2 * NKC * 2)]
    pat[5] = "A"  # one surplus ACT tile while the DVE does the v_aug adds
    return pat


def build_bass():
    import concourse.bass as bass
    import concourse.mybir as mybir
    import concourse.tile as tile

    _patch_drain()
    dt = mybir.dt
    f32 = dt.float32
    bf16 = dt.bfloat16
    i16 = dt.int16
    mmdt = bf16  # whole data path is bf16 (DMA bytes are the startup gate)
    AF = mybir.ActivationFunctionType
    OP = mybir.AluOpType

    nc = bass.Bass("TRN2", num_devices=NCORES, enable_asserts=True)

    kp_d = nc.dram_tensor("kp", [DM, 2 * S], mmdt, kind="ExternalInput")
    qp_d = nc.dram_tensor("qp", [DM, R], mmdt, kind="ExternalInput")
    # v ships in bf16 (it only feeds the bf16 v_aug path) with bf16 W2^T
    xv_d = nc.dram_tensor("xv", [DM, S + DM], bf16, kind="ExternalInput")
    # weights (w1T, w0Ts, w0T) + 4 f32 biases + v_aug template + WoT + identity
    WPW = 3 * DM + 8 + 32 * H + 2 * DM  # 904 bf16 columns
    wp_d = nc.dram_tensor("wp", [DM, WPW], mmdt, kind="ExternalInput")
    outT_d = nc.dram_tensor("outT", [DM, R], bf16, kind="ExternalOutput")

    ENG = _exp_engine_pattern()

    with tile.TileContext(nc) as tc:
        with (
            tc.tile_pool(name="singles", bufs=1) as singles,
            tc.tile_pool(name="exps", bufs=48) as exps,
            tc.tile_pool(name="tailp", bufs=2) as tailp,
        ):
            def loadt(name, shape, dtype):
                return singles.tile(shape, dtype, tag=name, name=name)

            s_w = loadt("w_s", [DM, WPW], mmdt)
            s_kp = loadt("kp_s", [DM, 2 * S], mmdt)
            s_qp = loadt("qp_s", [DM, R], mmdt)
            s_xv = loadt("xv_s", [DM, S + DM], bf16)
            # per head: [1*16 | v_h] -> the attn@v matmul also emits row sums
            # replicated over the first 16 columns of each 32-block
            v_aug = singles.tile([DM, NKC, 32 * H], bf16, tag="vaug", name="vaug")
            va4 = v_aug.rearrange("p t (h u) -> p t h u", u=32)
            # 512-col chunks, ordered so the projections gating the first
            # score tile (kT/pkT/pqT first halves + qT) unblock earliest.
            # xv is deferred until after the critical interleave DMAs (FIFO).
            def kp_chunk(c):
                nc.sync.dma_start(
                    out=s_kp[:, c * 1024 : (c + 1) * 1024],
                    in_=kp_d[:, c * 1024 : (c + 1) * 1024],
                )

            nc.sync.dma_start(out=s_w[:, :], in_=wp_d[:, :])
            kp_chunk(0)
            kp_chunk(2)
            nc.sync.dma_start(out=s_qp[:, 0:QB], in_=qp_d[:, 0:QB])
            nc.sync.dma_start(out=s_qp[:, QB:R], in_=qp_d[:, QB:R])
            kp_chunk(1)
            kp_chunk(3)

            w1T = s_w[:, 0:128]
            w0Ts = s_w[:, 128:256]
            w0T = s_w[:, 256:384]
            s_bias = s_w[:, 384:392].bitcast(f32)  # 4 f32 packed in 8 bf16 cols
            b1c = s_bias[:, 0:1]
            b0s = s_bias[:, 1:2]
            b0c = s_bias[:, 2:3]
            boc = s_bias[:, 3:4]
            # v_aug template [1*16 | b2_h] per head block
            vtpl = s_w[:, 392 : 392 + 32 * H]
            woT_b = s_w[:, 648:776]  # [128, 128] bf16 WoT (head-dense rows)
            identb = s_w[:, 776:904]  # [128, 128] bf16 identity

            kcat = [
                singles.tile([DM, S], mmdt, tag="kcatA", name="kcatA"),
                singles.tile([DM, S], mmdt, tag="kcatB", name="kcatB"),
            ]
            qcat = [
                singles.tile([DM, R], mmdt, tag="qcatA", name="qcatA"),
                singles.tile([DM, R], mmdt, tag="qcatB", name="qcatB"),
            ]

            ob = singles.tile([DM, R], bf16, tag="ob", name="ob")

            # ---------------- projections ----------------
            kT_sb = singles.tile([DM, S], mmdt, tag="kT_sb", name="kT_sb")
            pkT_sb = singles.tile([DM, S], mmdt, tag="pkT_sb", name="pkT_sb")
            qT_sb = singles.tile([DM, R], mmdt, tag="qT_sb", name="qT_sb")
            pqT_sb = singles.tile([DM, R], mmdt, tag="pqT_sb", name="pqT_sb")

            evac_flip = [0]

            def evac(dst_ap, src_ap, bias, force=None):
                # alternate evac between DVE and ACT so neither serializes;
                # late (in-loop) evacs are forced onto ACT — the DVE's serial
                # op chain is the loop's critical path
                use_act = force == "A" or (force is None and evac_flip[0] % 2 == 1)
                if use_act:
                    nc.scalar.activation(
                        out=dst_ap, in_=src_ap, func=AF.Identity, bias=bias
                    )
                else:
                    nc.vector.tensor_scalar_add(out=dst_ap, in0=src_ap, scalar1=bias)
                evac_flip[0] += 1

            def interleave(g, half, src_sb, dst, c0=0, c1=None, eng=None):
                # parity interleave: dst[g][2d + half] = src[64g + d]; a head
                # block is dst rows 32j..32j+32 (k/pos rows alternating — the
                # K=32 contraction is order-invariant). g0's interleaves are
                # column-chunked so the first score tiles start early; the
                # critical ones issue from different HWDGE queues in parallel.
                if c1 is None:
                    c1 = dst.shape[-1]
                dst_v = dst.rearrange("(d two) c -> d two c", two=2)[:, half, c0:c1]
                src_v = src_sb[64 * g : 64 * g + 64, c0:c1]
                (eng or nc.sync).dma_start(out=dst_v, in_=src_v)

            # PE p-state warmup on a memset tile: starts immediately (no DMA
            # dependency) so the ramp overlaps the input transfers and the
            # projections land at full clock
            warm_sb = singles.tile([DM, 512], mmdt, tag="warm_sb", name="warm_sb")
            nc.gpsimd.memset(warm_sb[:, :], 1.0)

            with tc.tile_pool(name="proj_psum", bufs=4, space="PSUM") as proj_psum:
                warm = proj_psum.tile([128, 512], f32, tag="warm", name="warm")
                for _ in range(4):
                    nc.tensor.matmul(
                        out=warm[:, :],
                        lhsT=warm_sb[:, 0:128],
                        rhs=warm_sb[:, :],
                        start=True,
                        stop=True,
                    )

                def projr(lhsT, rhs_src, c_lo, c_hi, bias, dst_sb, force=None):
                    for c0 in range(c_lo, c_hi, 512):
                        pk = proj_psum.tile([128, 512], f32, tag="proj", name="pk")
                        nc.tensor.matmul(
                            out=pk[:, :],
                            lhsT=lhsT,
                            rhs=rhs_src[:, c0 : c0 + 512],
                            start=True,
                            stop=True,
                        )
                        evac(dst_sb[:, c0 : c0 + 512], pk[:, :], bias, force)

                # c0 chunks of everything gating score tile 0 come first so
                # their evacs clear both engines earliest; the first kcat
                # interleave is split at the kc=0 boundary (tiny transfer)
                projr(w1T, s_kp[:, 0:S], 0, 512, b1c, kT_sb)
                projr(w1T, s_kp[:, S : 2 * S], 0, 512, b1c, pkT_sb)
                projr(w0Ts, s_qp, 0, 512, b0s, qT_sb)
                # pos_q rows sit at kp[:, S:S+R] thanks to the host key-roll
                projr(w0T, s_kp[:, S : S + R], 0, 512, b0c, pqT_sb)
                interleave(0, 0, qT_sb, qcat[0], 0, QB)
                interleave(0, 1, pqT_sb, qcat[0], 0, QB)
                interleave(0, 0, kT_sb, kcat[0], 0, 128)
                interleave(0, 1, pkT_sb, kcat[0], 0, 128)
                projr(w1T, s_kp[:, 0:S], 512, 1024, b1c, kT_sb)
                projr(w1T, s_kp[:, S : 2 * S], 512, 1024, b1c, pkT_sb)
                projr(w0Ts, s_qp, 512, R, b0s, qT_sb)
                projr(w0T, s_kp[:, S : S + R], 512, R, b0c, pqT_sb)
                interleave(0, 0, kT_sb, kcat[0], 128, 1024)
                interleave(0, 1, pkT_sb, kcat[0], 128, 1024)
                nc.sync.dma_start(out=s_xv[:, :], in_=xv_d[:, :])
                projr(w1T, s_kp[:, 0:S], 1024, 2048, b1c, kT_sb)
                projr(w1T, s_kp[:, S : 2 * S], 1024, 2048, b1c, pkT_sb)
                interleave(0, 0, kT_sb, kcat[0], 1024, 2048)
                interleave(0, 1, pkT_sb, kcat[0], 1024, 2048)
                interleave(0, 0, qT_sb, qcat[0], QB, R)
                interleave(0, 1, pqT_sb, qcat[0], QB, R)
                interleave(1, 0, kT_sb, kcat[1])
                interleave(1, 1, pkT_sb, kcat[1])
                interleave(1, 0, qT_sb, qcat[1])
                interleave(1, 1, pqT_sb, qcat[1])

            # ---------------- attention main loop ----------------
            # iter = (qc, g, kc, jj): head pair jj of group g, key chunk kc,
            # q block qc. kc outer of jj so each acc bank sees kc in order.
            ITERS = [
                (qc, g, kc, jj)
                for qc in range(NQB)
                for g in (0, 1)
                for kc in range(NKC)
                for jj in (0, 1)
            ]
            T = len(ITERS)
            sct = {}
            ets = {}
            accs = {}
            misc = {}

            with (
                tc.tile_pool(name="sc_psum", bufs=LOOK, space="PSUM") as sc_psum,
                tc.tile_pool(name="acc_psum", bufs=1, space="PSUM") as acc_psum,
                tc.tile_pool(name="misc_psum", bufs=1, space="PSUM") as misc_psum,
            ):
                # stamp the [1*16 | b2] template on the idle Pool engine
                for t in range(NKC):
                    nc.gpsimd.tensor_scalar_add(
                        out=v_aug[:, t, :], in0=vtpl, scalar1=0.0
                    )

                def emit_vgroup(g4):
                    # v projection, seq-major, 4 key-chunks batched into one
                    # PSUM bank. The bank is pre-filled with the b2 template
                    # via an identity matmul (start=True), the projections
                    # accumulate on top (start=False), and the evacuation is
                    # a plain ACT copy — keeping the DVE (the loop's critical
                    # serial chain) out of the v path entirely.
                    pv = acc_psum.tile([128, QB], f32, tag="accv", name="pv")
                    for c in range(4):
                        t = 4 * g4 + c
                        nc.tensor.matmul(
                            out=pv[:, 128 * c : 128 * (c + 1)],
                            lhsT=s_xv[:, t * 128 : (t + 1) * 128],
                            rhs=s_xv[:, S : S + DM],
                            start=(c == 0),
                            stop=(c == 3),
                        )
                    nc.vector.tensor_tensor(
                        out=va4[:, 4 * g4 : 4 * g4 + 4, :, 1:17],
                        in0=pv.rearrange("p (t h u) -> p t h u", t=4, u=16),
                        in1=va4[:, 4 * g4 : 4 * g4 + 4, :, 1:17],
                        op=OP.add,
                    )

                def emit_sc(t):
                    qc, g, kc, jj = ITERS[t]
                    st = sc_psum.tile([128, 2 * QB], f32, tag="sc", name="sc")
                    sct[t] = st
                    for i in (0, 1):
                        j = 2 * jj + i
                        nc.tensor.matmul(
                            out=st[:, QB * i : QB * (i + 1)],
                            lhsT=kcat[g][
                                32 * j : 32 * j + 32, kc * 128 : (kc + 1) * 128
                            ],
                            rhs=qcat[g][32 * j : 32 * j + 32, qc * QB : (qc + 1) * QB],
                            start=True,
                            stop=True,
                            tile_position=(32 * j, 0),
                        )

                def emit_exp(t):
                    e = exps.tile([128, 2 * QB], bf16, tag="e", name="e")
                    ets[t] = e

                    def act_exp(sl):
                        nc.scalar.activation(
                            out=e[:, sl], in_=sct[t][:, sl], func=AF.Exp
                        )

                    def dve_exp(sl):
                        nc.vector.tensor_scalar(
                            out=e.bitcast(i16)[:, sl],
                            in0=sct[t][:, sl],
                            scalar1=EXP_A,
                            scalar2=EXP_B,
                            op0=OP.mult,
                            op1=OP.add,
                        )

                    if ENG[t] == "A":
                        act_exp(slice(0, 2 * QB))
                    elif ENG[t] == "D":
                        dve_exp(slice(0, 2 * QB))
                    else:  # split tile: ACT head-half 0, DVE head-half 1
                        act_exp(slice(0, QB))
                        dve_exp(slice(QB, 2 * QB))
                    del sct[t]

                def emit_attnv(t):
                    # attn@v: e stationary, v_aug moving. Runs ATT_D tiles
                    # behind the exp stream so a stalled accumulator (waiting
                    # on the previous group's normalize) never blocks the
                    # score-tile refill in the in-order PE stream.
                    qc, g, kc, jj = ITERS[t]
                    if kc == 0 and jj == 0:
                        accs[(qc, g)] = acc_psum.tile(
                            [128, QB], f32, tag="accv", name="acc"
                        )
                    acc4 = accs[(qc, g)].rearrange(
                        "p (qch j u) -> p qch j u", qch=4, u=32
                    )
                    e = ets.pop(t)
                    for i in (0, 1):
                        j = 2 * jj + i
                        h = 4 * g + j
                        for qch in range(4):
                            nc.tensor.matmul(
                                out=acc4[:, qch, j, 0:17],
                                lhsT=e[:, QB * i + 128 * qch : QB * i + 128 * (qch + 1)],
                                rhs=v_aug[:, kc, 32 * h : 32 * h + 17],
                                start=(kc == 0 and jj == 0 and i == 0 and qch == 0),
                                stop=(kc == NKC - 1 and jj == 1 and i == 1 and qch == 3),
                                skip_group_check=True,
                            )
                    if kc == NKC - 1 and jj == 1:
                        emit_tail(qc, g, acc4)

                def emit_tail(qc, g, acc4):
                    # ---- (qc, g) done: normalize, transpose ----
                    rc = tailp.tile([DM, 16], f32, tag="rc", name="rc")
                    rc4 = rc.rearrange("p (qch j) -> p qch j", qch=4)
                    nc.vector.reciprocal(
                        out=rc4[:, :, :, None], in_=acc4[:, :, :, 0:1]
                    )
                    xs = tailp.tile([DM, 256], bf16, tag="xs", name="xs")
                    xs4 = xs.rearrange("p (qch j d) -> p qch j d", qch=4, d=16)
                    nc.vector.tensor_tensor(
                        out=xs4[:, :, :, :],
                        in0=acc4[:, :, :, 1:17],
                        in1=rc4[:, :, :, None].broadcast_to([DM, 4, 4, 16]),
                        op=OP.mult,
                    )
                    del accs[(qc, g)]

                    if g == 0:
                        misc[qc] = misc_psum.tile(
                            [128, QB], f32, tag="misc", name="pp"
                        )
                    pp = misc[qc]
                    ptr = pp.bitcast(bf16)  # [128, 1024] bf16 view
                    for qch in range(4):
                        nc.tensor.matmul(
                            out=ptr[64 * g : 64 * g + 64, 128 * qch : 128 * (qch + 1)],
                            lhsT=xs4[:, qch, :, :],
                            rhs=identb,
                            is_transpose=True,
                            start=(qch == 0),
                            stop=(qch == 3),
                            tile_position=(0, 64 * g),
                            skip_group_check=True,
                        )

                    if g == 1:
                        # ---- output projection, pipelined in halves after
                        # one full evac (po reuses pp's bank: its first write
                        # must come after ALL ptr reads in engine order) ----
                        xsT = tailp.tile([DM, QB], bf16, tag="xsT", name="xsT")
                        nc.scalar.activation(
                            out=xsT[:, :], in_=ptr[:, 0:QB], func=AF.Copy
                        )
                        po = misc_psum.tile([128, QB], f32, tag="misc", name="po")
                        del misc[qc]
                        for v2 in range(4):
                            hsl = slice(v2 * (QB // 4), (v2 + 1) * (QB // 4))
                            osl = slice(
                                qc * QB + v2 * (QB // 4),
                                qc * QB + (v2 + 1) * (QB // 4),
                            )
                            nc.tensor.matmul(
                                out=po[:, hsl],
                                lhsT=woT_b,
                                rhs=xsT[:, hsl],
                                start=(v2 == 0),
                                stop=(v2 == 3),
                                skip_group_check=True,
                            )
                            nc.scalar.activation(
                                out=ob[:, osl], in_=po[:, hsl],
                                func=AF.Identity, bias=boc,
                            )
                            nc.scalar.dma_start(out=outT_d[:, osl], in_=ob[:, osl])

                ATT_D = 8  # attn@v emission delay (tiles)
                for t in range(LOOK):
                    emit_sc(t)
                for t in range(T):
                    emit_exp(t)
                    if t + LOOK < T:
                        emit_sc(t + LOOK)
                    if t >= ATT_D:
                        emit_attnv(t - ATT_D)
                    if t < 4:
                        # v projections slot into the early loop: they wait
                        # on the xv DMA, which lands after the critical
                        # interleaves; all 4 must be allocated before the
                        # first acc tile (same 1-buf pool)
                        emit_vgroup(t)
                for t in range(T - ATT_D, T):
                    emit_attnv(t)

    _split_multi_waits(nc, mybir)
    return nc


def shard_inputs(query, key, value, pos_embed, W0, b0, W1, b1, W2, b2, Wo, bo):
    """Build the 8 per-core input maps (host-side layout preprocessing)."""
    f = np.float32
    asc = np.ascontiguousarray
    cat = np.concatenate
    scale = 1.0 / np.sqrt(np.float32(DK))

    import ml_dtypes

    bf16 = ml_dtypes.bfloat16

    # 4 f32 biases packed into 8 bf16 bit-container columns
    bias4 = asc(
        cat(
            [
                b1.astype(f).reshape(DM, 1),
                (b0 * scale).astype(f).reshape(DM, 1),
                b0.astype(f).reshape(DM, 1),
                bo.astype(f).reshape(DM, 1),
            ],
            axis=1,
        )
    ).view(np.uint16).view(bf16)

    # v_aug template: [ones(16) | b2_h(16)] per head block
    vtpl = np.zeros((DM, 32 * H), f)
    for h in range(H):
        vtpl[:, 32 * h] = 1.0
        vtpl[:, 32 * h + 1 : 32 * h + 17] = b2.astype(f)[None, 16 * h : 16 * h + 16]

    # WoT in head-dense row order (hd = h*16+d) — natural Wo.T
    woT = np.asarray(Wo).T.astype(f)
    ident = np.eye(DM, dtype=f)

    wp = asc(
        np.concatenate(
            [
                W1.T.astype(f).astype(bf16),
                (W0.T * scale).astype(f).astype(bf16),
                W0.T.astype(f).astype(bf16),
                bias4,
                vtpl.astype(bf16),
                woT.astype(bf16),
                ident.astype(bf16),
            ],
            axis=1,
        )
    )
    shared = {"wp": wp}
    in_maps = []
    for c in range(NCORES):
        b_i, half = divmod(c, 2)
        r0 = half * R
        # rotate the key axis by r0 (k/pos_k/v together — softmax and attn@v
        # are permutation-invariant over keys) so pos_q = pos rows r0..r0+R
        # sits at kp[:, S:S+R]
        perm = np.roll(np.arange(S), -r0)
        in_maps.append(
            dict(
                shared,
                kp=asc(
                    cat(
                        [key[b_i][perm].T, pos_embed[b_i][perm].T], axis=1
                    ).astype(f).astype(bf16)
                ),
                qp=asc(query[b_i, r0 : r0 + R, :].T.astype(f).astype(bf16)),
                xv=asc(cat([value[b_i][perm].T, W2.T], axis=1).astype(f).astype(bf16)),
            )
        )
    return in_maps


def gather_outputs(results):
    out = np.empty((B, S, DM), np.float32)
    for c in range(NCORES):
        b_i, half = divmod(c, 2)
        r0 = half * R
        out[b_i, r0 : r0 + R, :] = results[c]["outT"].T
    return out


def kernel(query, key, value, pos_embed, W0, b0, W1, b1, W2, b2, Wo, bo):
    from concourse.bass_utils import run_bass_kernel_spmd

    # inputs may arrive as jax arrays; materialize once so the host-side
    # slicing/transposing below stays in numpy
    args = [
        np.asarray(a)
        for a in (query, key, value, pos_embed, W0, b0, W1, b1, W2, b2, Wo, bo)
    ]
    if "nc" not in _CACHE:
        _CACHE["nc"] = build_bass()
    in_maps = shard_inputs(*args)
    res = run_bass_kernel_spmd(_CACHE["nc"], in_maps, core_ids=list(range(NCORES)))
    return gather_outputs(res.results)
